# revision 1
# baseline (speedup 1.0000x reference)
"""AffNetR TRN2 kernel: out[u,i] = ((max_h cos(Z[h,u,:], X[i,:])) + 1) / 2, ^beta.

Sharding: data-parallel over users (U=8192) across 8 NeuronCores; X replicated.
Each core computes a [1024, 8192] slice of the output.

v2: chunk-streamed prologue. Inputs arrive pre-transposed ([E=128, *]) and are
DMA'd in 512-column chunks (Z first). Per chunk: square on ACT (rounded to
float32r), column-sum-of-squares via an all-ones fp32r matmul into a PSUM row,
then per-round (4 chunks) the [1,2048] row bounces through an HBM scratch to
land partition-major as [4,512]. sqrt(+eps) + reciprocal (DVE) produce the
normalization scales; the X side folds the final /2 affine. One-hot selector
matmuls broadcast each chunk's scale row to 128 partitions, fused into the
operand during PSUM evacuation (DVE tensor_tensor multiply, float32r out).

Main loop per (u-tile 128, i-tile 512): 4 fp32r matmuls (one per head) into 4
PSUM banks; ACT evacuates head 0 adding the +0.5 affine bias; DVE folds heads
1-3 with fused scalar_tensor_tensor ops (out = (psum + 0.5) max chain);
gpsimd-issued DMAs store [128,2048] blocks.

A post-Tile pass splits excess semaphore waits onto inserted NoOps (several
ISA structs only accept one wait slot and walrus rejects more).
"""

import numpy as np

import concourse.bass as bass
import concourse.mybir as mybir
import concourse.tile as tile
from concourse.bass_utils import run_bass_kernel_spmd

F32 = mybir.dt.float32
F32R = mybir.dt.float32r

H = 4
U = 8192
E = 128
I = 8192
NCORES = 8
USH = U // NCORES          # 1024 users per core
UT = USH // 128            # 8 u-tiles
IT = I // 512              # 16 i-tiles
NXC = I // 512             # 16 x chunks of 512
NZC = (H * USH) // 512     # 8 z chunks of 512
EPS = 1e-6

_cache = {}


def _legalize_waits(nc, max_waits=1):
    """Hoist excess sem waits onto same-engine NoOps (1-wait ISA structs)."""
    cnt = 0
    for f in nc.m.functions:
        for blk in f.blocks:
            insts = blk.instructions
            out = []
            changed = False
            for inst in insts:
                si = inst.sync_info
                waits = list(si.on_wait) if si is not None and si.on_wait else []
                if len(waits) > max_waits and inst.engine is not None:
                    keep = waits[-max_waits:]
                    for w in waits[:-max_waits]:
                        nop = mybir.InstNoOp(name=f"wlg-{cnt}", ins=[], outs=[])
                        cnt += 1
                        nop.engine = inst.engine
                        nop.sync_info = mybir.SyncInfo(on_wait=[w], on_update=[])
                        out.append(nop)
                    upd = list(si.on_update) if si.on_update else []
                    inst.sync_info = mybir.SyncInfo(on_wait=keep, on_update=upd)
                    changed = True
                out.append(inst)
            if changed:
                blk.instructions = out
    return cnt


def _build():
    nc = bass.Bass()
    xt_d = nc.dram_tensor("xt", [E, I], F32, kind="ExternalInput")
    zt_d = nc.dram_tensor("zt", [E, H * USH], F32, kind="ExternalInput")
    sel_d = nc.dram_tensor("sel", [16, 16 * 128], F32R, kind="ExternalInput")
    out_d = nc.dram_tensor("out", [USH, I], F32, kind="ExternalOutput")
    scr_d = nc.dram_tensor("scr", [12, 1024], F32)
    out_v = out_d[:].rearrange("(uo p) i -> p uo i", p=128)

    S = mybir.ActivationFunctionType

    with tile.TileContext(nc) as tc:
        with tc.tile_pool(name="big", bufs=1) as big:
            pre_ctx = tc.tile_pool(name="pre", bufs=1)
            pre = pre_ctx.__enter__()
            xt_sb = pre.tile([E, I], F32, tag="xt_sb")
            zt_sb = pre.tile([E, H * USH], F32, tag="zt_sb")
            sqx = pre.tile([E, I], F32R, tag="sqx")
            sqz = pre.tile([E, H * USH], F32R, tag="sqz")
            rxg = pre.tile([16, 512], F32, tag="rxg")
            rzg = pre.tile([8, 512], F32, tag="rzg")
            sel_r = big.tile([16, 16 * 128], F32R, tag="sel_r")
            xtn = big.tile([E, I], F32R, tag="xtn")
            ztn = big.tile([E, H * USH], F32R, tag="ztn")
            rx05 = big.tile([16, 512], F32R, tag="rx05")
            rz1 = big.tile([8, 512], F32R, tag="rz1")

            # input DMAs, Z chunks first (Z gates the main loop's lhsT)
            for c in range(NZC):
                s = slice(c * 512, (c + 1) * 512)
                nc.sync.dma_start(zt_sb[:, s], zt_d[:, s])
            for c in range(NXC):
                s = slice(c * 512, (c + 1) * 512)
                nc.sync.dma_start(xt_sb[:, s], xt_d[:, s])
            nc.sync.dma_start(sel_r, sel_d[:])

            onesf = big.tile([128, 1], F32, tag="onesf")
            nc.vector.memset(onesf, 1.0)
            ones_r = big.tile([128, 1], F32R, tag="ones_r")
            nc.scalar.copy(ones_r, onesf)
            half1 = big.tile([128, 1], F32, tag="half1")
            nc.vector.memset(half1, 0.5)

            rows_ctx = tc.tile_pool(name="rows", bufs=4)
            rows_pool = rows_ctx.__enter__()
            pcols_ctx = tc.tile_pool(name="pcols", bufs=2, space="PSUM")
            pcols = pcols_ctx.__enter__()
            prep_ctx = tc.tile_pool(name="prep", bufs=4, space="PSUM")
            prep = prep_ctx.__enter__()

            def colsum_rounds(src, sq, nchunks, scr0, scat):
                """Square chunks, column-sum via all-ones matmul, bounce each
                [1,1024] round through HBM to land as [2,512] partition rows."""
                for rnd in range(nchunks // 2):
                    ss = pcols.tile([1, 1024], F32, tag="ss")
                    for j in range(2):
                        c = rnd * 2 + j
                        s = slice(c * 512, (c + 1) * 512)
                        nc.scalar.activation(sq[:, s], src[:, s], S.Square)
                        nc.tensor.matmul(
                            ss[:, j * 512 : (j + 1) * 512],
                            ones_r,
                            sq[:, s],
                            start=True,
                            stop=True,
                        )
                    row = rows_pool.tile([1, 1024], F32, tag="row")
                    nc.vector.tensor_copy(row, ss)
                    k = scr0 + rnd
                    nc.gpsimd.dma_start(scr_d[k : k + 1, :], row)
                    nc.sync.dma_start(
                        scat[rnd * 2 : (rnd + 1) * 2, :],
                        scr_d[k, :].rearrange("(c n) -> c n", c=2),
                    )

            def norm_chain(g, n_par, scale, out_r):
                nrm = pre.tile([n_par, 512], F32, tag=f"nrm{n_par}")
                nc.scalar.activation(nrm, g, S.Sqrt)
                ne = pre.tile([n_par, 512], F32, tag=f"ne{n_par}")
                nc.vector.tensor_scalar_add(ne, nrm, EPS)
                rr = pre.tile([n_par, 512], F32, tag=f"rr{n_par}")
                nc.vector.reciprocal(rr, ne)
                nc.vector.tensor_scalar_mul(out_r, rr, scale)

            def replicate_evac(cs, kk, r_in, src, dst):
                for c in cs:
                    s = slice(c * 512, (c + 1) * 512)
                    rep = prep.tile([128, 512], F32, tag="rep")
                    nc.tensor.matmul(
                        rep,
                        sel_r[0:kk, c * 128 : (c + 1) * 128],
                        r_in,
                        start=True,
                        stop=True,
                    )
                    nc.vector.tensor_tensor(
                        dst[:, s], src[:, s], rep, mybir.AluOpType.mult
                    )

            # colsums first, then chains, then replicates (v5 order)
            colsum_rounds(zt_sb, sqz, NZC, 0, rzg)
            colsum_rounds(xt_sb, sqx, NXC, 4, rxg)
            norm_chain(rzg, 8, 1.0, rz1)
            norm_chain(rxg, 16, 0.5, rx05)
            replicate_evac(range(NZC), 8, rz1, zt_sb, ztn)
            replicate_evac(range(NXC), 16, rx05, xt_sb, xtn)

            prep_ctx.__exit__(None, None, None)
            pcols_ctx.__exit__(None, None, None)
            rows_ctx.__exit__(None, None, None)
            pre_ctx.__exit__(None, None, None)

            # ---------- main loop ----------
            with (
                tc.tile_pool(name="work", bufs=3) as work,
                tc.tile_pool(name="ost", bufs=2) as ost,
                tc.tile_pool(name="pmm", bufs=2, space="PSUM") as pmm,
            ):
                for ut in range(UT):
                    lhs = [
                        ztn[:, h * USH + ut * 128 : h * USH + (ut + 1) * 128]
                        for h in range(H)
                    ]
                    for it in range(IT):
                        rhs = xtn[:, it * 512 : (it + 1) * 512]
                        ps = []
                        for h in range(H):
                            p = pmm.tile([128, 512], F32, tag=f"p{h}")
                            nc.tensor.matmul(p, lhs[h], rhs, start=True, stop=True)
                            ps.append(p)
                        c0 = work.tile([128, 512], F32, tag="c0")
                        nc.scalar.activation(
                            c0, ps[0], S.Identity, bias=half1, scale=1.0
                        )
                        m1 = work.tile([128, 512], F32, tag="m1")
                        nc.vector.scalar_tensor_tensor(
                            m1, ps[1], 0.5, c0,
                            op0=mybir.AluOpType.add, op1=mybir.AluOpType.max,
                        )
                        m2 = work.tile([128, 512], F32, tag="m2")
                        nc.vector.scalar_tensor_tensor(
                            m2, ps[2], 0.5, m1,
                            op0=mybir.AluOpType.add, op1=mybir.AluOpType.max,
                        )
                        if it % 4 == 0:
                            ostage = ost.tile([128, 2048], F32, tag="ostage")
                        nc.vector.scalar_tensor_tensor(
                            ostage[:, (it % 4) * 512 : (it % 4 + 1) * 512],
                            ps[3], 0.5, m2,
                            op0=mybir.AluOpType.add, op1=mybir.AluOpType.max,
                        )
                        if ut == UT - 1 and it >= 12:
                            j = it % 4
                            nc.gpsimd.dma_start(
                                out_v[:, ut, (12 + j) * 512 : (13 + j) * 512],
                                ostage[:, j * 512 : (j + 1) * 512],
                            )
                        elif it % 4 == 3:
                            ig = it // 4
                            nc.gpsimd.dma_start(
                                out_v[:, ut, ig * 2048 : (ig + 1) * 2048],
                                ostage,
                            )

    _legalize_waits(nc)
    return nc


def _sel_host():
    sel = np.zeros((16, 16 * 128), dtype=np.float32)
    for c in range(16):
        sel[c, c * 128 : (c + 1) * 128] = 1.0
    return sel


def kernel(X, Z, beta):
    X = np.asarray(X, dtype=np.float32)
    Z = np.asarray(Z, dtype=np.float32)
    xt = np.ascontiguousarray(X.T)                      # [128, 8192]
    sel = _sel_host()
    in_maps = []
    for c in range(NCORES):
        zs = Z[:, c * USH : (c + 1) * USH, :]           # [4, 1024, 128]
        zt = np.ascontiguousarray(
            zs.transpose(2, 0, 1).reshape(E, H * USH)
        )                                               # [128, 4096]
        in_maps.append({"xt": xt, "zt": zt, "sel": sel})

    if "nc" not in _cache:
        _cache["nc"] = _build()
    res = run_bass_kernel_spmd(_cache["nc"], in_maps, list(range(NCORES))).results
    out = np.concatenate([r["out"] for r in res], axis=0)

    b = float(np.asarray(beta))
    if b != 1.0:
        out = np.power(out, b).astype(np.float32)
    return out



# revision 7
# speedup vs baseline: 1.9188x; 1.9188x over previous
"""AffNetR TRN2 kernel v3: out[u,i] = ((max_h cos(Z[h,u,:], X[i,:])) + 1) / 2, ^beta.

Sharding: data-parallel over users (U=8192) across 8 NeuronCores; X replicated.
Each core computes a [1024, 8192] slice of the output.

Normalization is folded host-side into bf16 inputs. The cross-head max uses
max(a,b) = (a+b)/2 + |a-b|/2 on sum/diff stationaries (za01 = (z0n+z1n)/4,
zd01 = (z0n-z1n)/4, same for heads 2,3), so per 512-col tile the PSUM holds
[s01|s23] (2 banks) and [d01|d23] (2 banks):
  ACT: t = Abs([d01|d23])  -> SBUF f32, one fs1024 op (2 banks)
  DVE: m = [s01|s23] + t   -> SBUF bf16, one fs1024 op (PSUM is ONE operand;
        TRN2 allows only one PSUM input per DVE instruction)
giving m = [max(c0,c1)|max(c2,c3)] / 1 with c_h = cos_h/2.

The final cross-pair max runs on-device (FINAL_ENG) or on the host
(FINAL_ENG="host": kernel DMAs the interleaved pair-max rows and numpy
does maximum + affine, which keeps DVE at one op/tile).
"""

import numpy as np
import ml_dtypes

import concourse.bass as bass
import concourse.mybir as mybir
import concourse.tile as tile
from concourse.bass_utils import run_bass_kernel_spmd

F32 = mybir.dt.float32
BF16 = mybir.dt.bfloat16
BF16_NP = ml_dtypes.bfloat16

H = 4
U = 8192
E = 128
I = 8192
NCORES = 8
USH = U // NCORES          # 1024 users per core
UT = USH // 128            # 8 u-tiles
IT = I // 512              # 16 i-tiles of 512
ZCOLS = 4 * 128            # per-ut stationary pack: za01|zd01|za23|zd23
EPS = 1e-6

# ---- tunables -------------------------------------------------------------
FINAL_ENG = "host"         # "host" | "gp" | "dve"
# ---------------------------------------------------------------------------

_cache = {}


def _legalize_waits(nc, max_waits=1):
    """Hoist excess sem waits onto same-engine NoOps (1-wait ISA structs)."""
    cnt = 0
    for f in nc.m.functions:
        for blk in f.blocks:
            insts = blk.instructions
            out = []
            changed = False
            for inst in insts:
                si = inst.sync_info
                waits = list(si.on_wait) if si is not None and si.on_wait else []
                if len(waits) > max_waits and inst.engine is not None:
                    keep = waits[-max_waits:]
                    for w in waits[:-max_waits]:
                        nop = mybir.InstNoOp(name=f"wlg-{cnt}", ins=[], outs=[])
                        cnt += 1
                        nop.engine = inst.engine
                        nop.sync_info = mybir.SyncInfo(on_wait=[w], on_update=[])
                        out.append(nop)
                    upd = list(si.on_update) if si.on_update else []
                    inst.sync_info = mybir.SyncInfo(on_wait=keep, on_update=upd)
                    changed = True
                out.append(inst)
            if changed:
                blk.instructions = out
    return cnt


def _build(legalize=True):
    host_final = FINAL_ENG == "host"
    nc = bass.Bass()
    xs_d = nc.dram_tensor("xs", [E, I], BF16, kind="ExternalInput")
    zp_d = nc.dram_tensor("zp", [E, UT * ZCOLS], BF16, kind="ExternalInput")
    ow = 2 * I if host_final else I
    out_d = nc.dram_tensor("out", [USH, ow], BF16, kind="ExternalOutput")
    out_v = out_d[:].rearrange("(uo p) i -> p uo i", p=128)

    S = mybir.ActivationFunctionType
    A = mybir.AluOpType

    with tile.TileContext(nc) as tc:
        with (
            tc.tile_pool(name="cst", bufs=1) as cst,
            tc.tile_pool(name="tb", bufs=3) as tb,
            tc.tile_pool(name="rows", bufs=2) as rows,
            tc.tile_pool(name="ost", bufs=2) as ost,
            tc.tile_pool(name="ps", bufs=2, space="PSUM") as ps,
            tc.tile_pool(name="pd", bufs=2, space="PSUM") as pd,
        ):
            zp_sb = cst.tile([E, UT * ZCOLS], BF16, tag="zp_sb", name="zp_sb")
            xs_sb = cst.tile([E, I], BF16, tag="xs_sb", name="xs_sb")
            for u in range(UT):
                s = slice(u * ZCOLS, (u + 1) * ZCOLS)
                nc.sync.dma_start(zp_sb[:, s], zp_d[:, s])
            for c in range(I // 512):
                s = slice(c * 512, (c + 1) * 512)
                nc.sync.dma_start(xs_sb[:, s], xs_d[:, s])

            for ut in range(UT):
                zb = ut * ZCOLS
                za01 = zp_sb[:, zb + 0 * 128 : zb + 1 * 128]
                zd01 = zp_sb[:, zb + 1 * 128 : zb + 2 * 128]
                za23 = zp_sb[:, zb + 2 * 128 : zb + 3 * 128]
                zd23 = zp_sb[:, zb + 3 * 128 : zb + 4 * 128]
                for blk in range(IT // 4):
                    mrow = rows.tile([128, 4096], BF16, tag="mrow", name="mrow")
                    for j in range(4):
                        it = blk * 4 + j
                        xv = xs_sb[:, it * 512 : (it + 1) * 512]
                        gs = ps.tile([128, 1024], F32, tag="gs", name="gs")
                        gd = pd.tile([128, 1024], F32, tag="gd", name="gd")
                        nc.tensor.matmul(gs[:, 0:512], za01, xv, start=True, stop=True)
                        nc.tensor.matmul(gs[:, 512:1024], za23, xv, start=True, stop=True)
                        nc.tensor.matmul(gd[:, 0:512], zd01, xv, start=True, stop=True)
                        nc.tensor.matmul(gd[:, 512:1024], zd23, xv, start=True, stop=True)

                        t = tb.tile([128, 1024], F32, tag="t", name="t")
                        nc.scalar.activation(t, gd, S.Abs)
                        # m = [m01|m23] = s + |d|  (PSUM is in0, single operand)
                        nc.vector.tensor_tensor(
                            mrow[:, j * 1024 : (j + 1) * 1024], gs, t, A.add
                        )

                    if host_final:
                        nc.gpsimd.dma_start(
                            out_v[:, ut, blk * 4096 : (blk + 1) * 4096], mrow
                        )
                    else:
                        ostage = ost.tile([128, 2048], BF16, tag="ostage", name="ostage")
                        mv = mrow.rearrange("p (b two c) -> p b two c", two=2, c=512)
                        eng = nc.gpsimd if FINAL_ENG == "gp" else nc.vector
                        ov = ostage.rearrange("p (b c) -> p b c", c=512)
                        eng.tensor_tensor(ov, mv[:, :, 0, :], mv[:, :, 1, :], A.max)
                        nc.gpsimd.dma_start(
                            out_v[:, ut, blk * 2048 : (blk + 1) * 2048], ostage
                        )

    if legalize:
        _legalize_waits(nc)
    return nc


def _prep_inputs(X, Z):
    X = np.asarray(X, dtype=np.float32)
    Z = np.asarray(Z, dtype=np.float32)
    xn = np.linalg.norm(X, axis=1) + EPS                    # [I]
    xs = np.ascontiguousarray((X / xn[:, None]).T)          # [128, I] unit rows
    xs = xs.astype(BF16_NP)

    zn = np.linalg.norm(Z, axis=2) + EPS                    # [H, U]
    Zs = Z / zn[:, :, None]                                 # [H, U, 128] unit
    in_maps = []
    for c in range(NCORES):
        zc = Zs[:, c * USH : (c + 1) * USH, :]              # [4, 1024, 128]
        zp = np.empty((E, UT * ZCOLS), dtype=np.float32)
        for ut in range(UT):
            us = slice(ut * 128, (ut + 1) * 128)
            z0 = zc[0, us].T                                # [128e, 128u]
            z1 = zc[1, us].T
            z2 = zc[2, us].T
            z3 = zc[3, us].T
            zb = ut * ZCOLS
            zp[:, zb + 0 * 128 : zb + 1 * 128] = (z0 + z1) * 0.25
            zp[:, zb + 1 * 128 : zb + 2 * 128] = (z0 - z1) * 0.25
            zp[:, zb + 2 * 128 : zb + 3 * 128] = (z2 + z3) * 0.25
            zp[:, zb + 3 * 128 : zb + 4 * 128] = (z2 - z3) * 0.25
        in_maps.append({"xs": xs, "zp": zp.astype(BF16_NP)})
    return in_maps


def kernel(X, Z, beta):
    in_maps = _prep_inputs(X, Z)
    if "nc" not in _cache:
        _cache["nc"] = _build()
    res = run_bass_kernel_spmd(_cache["nc"], in_maps, list(range(NCORES))).results

    out = np.empty((U, I), dtype=np.float32)
    for c in range(NCORES):
        m = res[c]["out"]
        if FINAL_ENG == "host":
            # cols: blocks of 4096 = 4 tiles x [m01(512)|m23(512)]
            m4 = m.reshape(USH, I // 512, 2, 512).astype(np.float32)
            out[c * USH : (c + 1) * USH] = np.maximum(
                m4[:, :, 0, :], m4[:, :, 1, :]
            ).reshape(USH, I)
        else:
            out[c * USH : (c + 1) * USH] = m.astype(np.float32)
    out += 0.5

    b = float(np.asarray(beta))
    if b != 1.0:
        out = np.power(out, b).astype(np.float32)
    return out
